# revision 1
# baseline (speedup 1.0000x reference)
"""HSTU block kernel for 8 Trainium2 NeuronCores.

Sharding: token-parallel. Core c handles batch b=c//4, tokens
[(c%4)*512, (c%4+1)*512). f1/attention/LN/f2 all computed locally for the
core's 512 query tokens; k/v for the full 2048-token batch are exchanged
with one AllGather per 4-core group.

Dataflow is feature-major (features on partitions) so the only transposes
are the initial x -> xT (32 PE transposes). LayerNorm over the feature dim
uses a ones-column matmul for the partition reduction and a K=1 ones-row
matmul to broadcast per-token stats back across partitions. The reference's
silu(scores)/S scaling is folded into LayerNorm via eps' = S^2 * eps
(LN is scale-invariant except for eps).

All big matmuls run in float32r (~13-bit mantissa, full PE rate).
"""

import sys

sys.path.insert(0, "/opt/trn_rl_repo")

import ml_dtypes
import numpy as np

import concourse.bass as bass
import concourse.mybir as mybir
import concourse.tile as tile
from concourse import bacc
from concourse.bass_utils import run_bass_kernel_spmd
from concourse.masks import make_identity

F32 = mybir.dt.float32
F32R = mybir.dt.float32r
BF16 = mybir.dt.bfloat16
SILU = mybir.ActivationFunctionType.Silu
SQRT = mybir.ActivationFunctionType.Sqrt
MULT = mybir.AluOpType.mult
ADD = mybir.AluOpType.add
SUB = mybir.AluOpType.subtract

B, S, D = 2, 2048, 1024
H, DH = 16, 64
T = 512            # tokens per core
NT = T // 128      # 4 token tiles per core
KC = D // 128      # 8 contraction chunks
NP = 8             # head pairs
EPS_EFF = float(S) * float(S) * 1e-5

_CACHE = {}


def _build():
    nc = bacc.Bacc(None, target_bir_lowering=False, num_devices=8)

    x_s = nc.dram_tensor("x_s", [T, D], F32, kind="ExternalInput")
    W1 = nc.dram_tensor("W1", [D, 4 * D], F32R, kind="ExternalInput")
    b1 = nc.dram_tensor("b1", [4 * D], F32, kind="ExternalInput")
    W2 = nc.dram_tensor("W2", [D, D], F32R, kind="ExternalInput")
    b2 = nc.dram_tensor("b2", [D], F32R, kind="ExternalInput")
    gamma = nc.dram_tensor("gamma", [D], F32, kind="ExternalInput")
    beta = nc.dram_tensor("beta", [D], F32, kind="ExternalInput")
    y_s = nc.dram_tensor("y_s", [T, D], F32, kind="ExternalOutput")

    # W1 column blocks: u [0:D], v [D:2D], q [2D:3D], k [3D:4D]
    U0, V0, Q0, K0 = 0, D, 2 * D, 3 * D

    with tile.TileContext(nc) as tc:
        with (
            tc.tile_pool(name="persist", bufs=1) as sbp,
            tc.tile_pool(name="small", bufs=2) as sbs,
            tc.tile_pool(name="dram", bufs=1, space="DRAM") as dram,
        ):
            # ---- constants
            ident = sbp.tile([128, 128], F32)
            make_identity(nc, ident[:])
            ones_f = sbp.tile([128, 128], F32)
            nc.vector.memset(ones_f[:], 1.0)
            ones_col = sbp.tile([128, 1], F32R)
            nc.vector.tensor_copy(ones_col[:], ones_f[:, 0:1])
            ones_row = sbp.tile([1, 128], F32R)
            nc.vector.tensor_copy(ones_row[:], ones_f[0:1, :])

            b1q = sbp.tile([128, 8], F32)
            b1k = sbp.tile([128, 8], F32)
            b1u = sbp.tile([128, 8], F32)
            nc.sync.dma_start(b1q[:], b1[Q0:Q0 + D].rearrange("(c p) -> p c", p=128))
            nc.sync.dma_start(b1k[:], b1[K0:K0 + D].rearrange("(c p) -> p c", p=128))
            nc.sync.dma_start(b1u[:], b1[U0:U0 + D].rearrange("(c p) -> p c", p=128))
            gam = sbp.tile([128, 8], F32)
            bet = sbp.tile([128, 8], F32)
            nc.sync.dma_start(gam[:], gamma[:].rearrange("(c p) -> p c", p=128))
            nc.sync.dma_start(bet[:], beta[:].rearrange("(c p) -> p c", p=128))

            b1v_row = sbp.tile([1, D], F32R)
            nc.sync.dma_start(b1v_row[:], b1[V0:V0 + D][None, :].bitcast(F32R))
            b2_row = sbp.tile([1, D], F32R)
            nc.sync.dma_start(b2_row[:], b2[:][None, :])

            # broadcast b1v / b2 across partitions via K=1 ones matmul
            b1v_sb = sbp.tile([128, D], F32)
            b2_sb = sbp.tile([128, D], F32)
            with tc.tile_pool(name="ps_bc", bufs=2, space="PSUM") as ps_bc:
                for nf in range(2):
                    pb = ps_bc.tile([128, 512], F32, tag="bc")
                    nc.tensor.matmul(pb[:], ones_row[:], b1v_row[:, nf * 512:(nf + 1) * 512],
                                     start=True, stop=True)
                    nc.vector.tensor_copy(b1v_sb[:, nf * 512:(nf + 1) * 512], pb[:])
                for nf in range(2):
                    pb = ps_bc.tile([128, 512], F32, tag="bc")
                    nc.tensor.matmul(pb[:], ones_row[:], b2_row[:, nf * 512:(nf + 1) * 512],
                                     start=True, stop=True)
                    nc.vector.tensor_copy(b2_sb[:, nf * 512:(nf + 1) * 512], pb[:])

            # ---- persistent activations
            xT = sbp.tile([128, KC, T], F32R)        # x^T, d on partitions
            qT = sbp.tile([128, NP, T], BF16)
            uT = sbp.tile([128, NP, T], F32)
            gatedT = sbp.tile([128, KC, T], F32R)
            normedT = sbp.tile([128, KC, T], F32R)

            # AG bounce buffers
            kv_in = dram.tile([128, 16, T], BF16)
            kv_out = dram.tile([512, 16, T], BF16)

            # ================= stage 0: load + transpose x =================
            with (
                tc.tile_pool(name="xload", bufs=2) as xload,
                tc.tile_pool(name="ps_tr", bufs=4, space="PSUM") as ps_tr,
            ):
                for tt in range(NT):
                    xa = xload.tile([128, D], F32, tag="xa")
                    nc.sync.dma_start(xa[:], x_s[tt * 128:(tt + 1) * 128, :])
                    for kc in range(KC):
                        pt = ps_tr.tile([128, 128], F32, tag="tr")
                        nc.tensor.transpose(pt[:], xa[:, kc * 128:(kc + 1) * 128], ident[:])
                        nc.vector.tensor_copy(xT[:, kc, tt * 128:(tt + 1) * 128], pt[:])

            # ================= stage 1: f1 =================
            with (
                tc.tile_pool(name="w1pool", bufs=12) as w1pool,
                tc.tile_pool(name="wvpool", bufs=2) as wvpool,
                tc.tile_pool(name="kvloc", bufs=1) as kvloc,
            ):
                kT_loc = kvloc.tile([128, NP, T], BF16)
                v_loc = kvloc.tile([128, NT, D], BF16)

                # k (feature-major) -> kT_loc
                with tc.tile_pool(name="ps_k", bufs=2, space="PSUM") as ps_k:
                  for hc in range(NP):
                    ps = ps_k.tile([128, T], F32, tag="f1")
                    for kc in range(KC):
                        wb = w1pool.tile([128, 128], F32R, tag="w1blk")
                        nc.sync.dma_start(
                            wb[:], W1[kc * 128:(kc + 1) * 128, K0 + hc * 128:K0 + (hc + 1) * 128])
                        nc.tensor.matmul(ps[:], wb[:], xT[:, kc, :],
                                         start=(kc == 0), stop=(kc == KC - 1))
                    nc.scalar.activation(kT_loc[:, hc, :], ps[:], SILU,
                                         bias=b1k[:, hc:hc + 1], scale=1.0)
                nc.gpsimd.dma_start(kv_in[:, 0:8, :], kT_loc[:])

                # v (token-major) -> v_loc; kc outer so each xT lhsT load
                # feeds both nf matmuls
                with tc.tile_pool(name="ps_v", bufs=1, space="PSUM") as ps_v:
                  psv = [ps_v.tile([128, 1024], F32, tag=f"v{tt}", name=f"psv{tt}")
                         for tt in range(NT)]
                  for kc in range(KC):
                    wv = wvpool.tile([128, 1024], F32R, tag="wv")
                    nc.sync.dma_start(wv[:], W1[kc * 128:(kc + 1) * 128, V0:V0 + D])
                    for tt in range(NT):
                        for nf in range(2):
                            nc.tensor.matmul(psv[tt][:, nf * 512:(nf + 1) * 512],
                                             xT[:, kc, tt * 128:(tt + 1) * 128],
                                             wv[:, nf * 512:(nf + 1) * 512],
                                             start=(kc == 0), stop=(kc == KC - 1))
                  for tt in range(NT):
                    vt = sbs.tile([128, 1024], F32, tag="vtmp")
                    nc.vector.tensor_tensor(vt[:], psv[tt][:], b1v_sb[:], ADD)
                    nc.scalar.activation(v_loc[:, tt, :], vt[:], SILU)
                nc.gpsimd.dma_start(
                    kv_in[:, 8:16, :],
                    v_loc[:].rearrange("p tt (h f) -> p (tt h) f", h=2))
                tc.no_sync_barrier()

                # q, u (overlap the AllGather)
                with tc.tile_pool(name="ps_qu", bufs=2, space="PSUM") as ps_qu:
                  for hc in range(NP):
                    ps = ps_qu.tile([128, T], F32, tag="f1")
                    for kc in range(KC):
                        wb = w1pool.tile([128, 128], F32R, tag="w1blk")
                        nc.sync.dma_start(
                            wb[:], W1[kc * 128:(kc + 1) * 128, Q0 + hc * 128:Q0 + (hc + 1) * 128])
                        nc.tensor.matmul(ps[:], wb[:], xT[:, kc, :],
                                         start=(kc == 0), stop=(kc == KC - 1))
                    nc.scalar.activation(qT[:, hc, :], ps[:], SILU,
                                         bias=b1q[:, hc:hc + 1], scale=1.0)
                  for hc in range(NP):
                    ps = ps_qu.tile([128, T], F32, tag="f1")
                    for kc in range(KC):
                        wb = w1pool.tile([128, 128], F32R, tag="w1blk")
                        nc.sync.dma_start(
                            wb[:], W1[kc * 128:(kc + 1) * 128, U0 + hc * 128:U0 + (hc + 1) * 128])
                        nc.tensor.matmul(ps[:], wb[:], xT[:, kc, :],
                                         start=(kc == 0), stop=(kc == KC - 1))
                    nc.scalar.activation(uT[:, hc, :], ps[:], SILU,
                                         bias=b1u[:, hc:hc + 1], scale=1.0)

                # single AllGather for k+v within each 4-core group
                nc.gpsimd.collective_compute(
                    "AllGather", mybir.AluOpType.bypass,
                    replica_groups=[[0, 1, 2, 3], [4, 5, 6, 7]],
                    ins=[kv_in[:]], outs=[kv_out[:]])

            # ================= stage 2: attention per head pair =================
            with (
                tc.tile_pool(name="kvfull", bufs=2) as kvfull,
                tc.tile_pool(name="attn", bufs=2) as attn,
                tc.tile_pool(name="ps_s", bufs=1, space="PSUM") as ps_s,
                tc.tile_pool(name="ps_av", bufs=2, space="PSUM") as ps_av,
            ):
                for hc in range(NP):
                    ktf = kvfull.tile([128, 2048], BF16, tag="ktf")
                    for r in range(4):
                        nc.sync.dma_start(ktf[:, r * 512:(r + 1) * 512],
                                          kv_out[r * 128:(r + 1) * 128, hc, :])
                    vf = kvfull.tile([128, 16, 128], BF16, tag="vf")
                    for r in range(4):
                        for tt in range(NT):
                            nc.sync.dma_start(
                                vf[:, r * 4 + tt, :],
                                kv_out[r * 128:(r + 1) * 128, 8 + tt * 2 + hc // 4,
                                       (hc % 4) * 128:(hc % 4) * 128 + 128])

                    av0 = ps_av.tile([128, 512], F32, tag="av0")
                    av1 = ps_av.tile([128, 512], F32, tag="av1")
                    for kg in range(8):
                        s0 = ps_s.tile([128, 1024], F32, tag="s0")
                        s1 = ps_s.tile([128, 1024], F32, tag="s1")
                        for sub in range(2):
                            ktc = kg * 2 + sub
                            nc.tensor.matmul(s0[:, sub * 512:(sub + 1) * 512],
                                             ktf[0:64, ktc * 128:(ktc + 1) * 128],
                                             qT[0:64, hc, :], start=True, stop=True)
                            nc.tensor.matmul(s1[:, sub * 512:(sub + 1) * 512],
                                             ktf[64:128, ktc * 128:(ktc + 1) * 128],
                                             qT[64:128, hc, :], start=True, stop=True,
                                             tile_position=(64, 0))
                        a0 = attn.tile([128, 1024], BF16, tag="a0")
                        a1 = attn.tile([128, 1024], BF16, tag="a1")
                        nc.scalar.activation(a0[:], s0[:], SILU)
                        nc.scalar.activation(a1[:], s1[:], SILU)
                        for sub in range(2):
                            ktc = kg * 2 + sub
                            # full-width lhsT: head0 valid rows 0:64, head1 rows 64:128
                            nc.tensor.matmul(av0[:], vf[:, ktc, :],
                                             a0[:, sub * 512:(sub + 1) * 512],
                                             start=(ktc == 0), stop=(ktc == 15))
                            nc.tensor.matmul(av1[:], vf[:, ktc, :],
                                             a1[:, sub * 512:(sub + 1) * 512],
                                             start=(ktc == 0), stop=(ktc == 15))
                    nc.vector.tensor_tensor(gatedT[0:64, hc, :], av0[0:64, :],
                                            uT[0:64, hc, :], MULT)
                    nc.vector.tensor_tensor(gatedT[64:128, hc, :], av1[64:128, :],
                                            uT[64:128, hc, :], MULT)

            # ================= stage 3: LayerNorm =================
            with (
                tc.tile_pool(name="ln", bufs=2) as ln,
                tc.tile_pool(name="ps_ln", bufs=1, space="PSUM") as ps_ln,
            ):
                st_sum = ps_ln.tile([1, T], F32, tag="st_sum")
                st_sq = ps_ln.tile([1, T], F32, tag="st_sq")
                for kc in range(KC):
                    nc.tensor.matmul(st_sum[:], ones_col[:], gatedT[:, kc, :],
                                     start=(kc == 0), stop=(kc == KC - 1))
                for kc in range(KC):
                    sq = ln.tile([128, T], F32R, tag="sq")
                    nc.vector.tensor_tensor(sq[:], gatedT[:, kc, :].bitcast(F32),
                                            gatedT[:, kc, :].bitcast(F32), MULT)
                    nc.tensor.matmul(st_sq[:], ones_col[:], sq[:],
                                     start=(kc == 0), stop=(kc == KC - 1))

                mu = ln.tile([1, T], F32, tag="mu")
                nc.vector.tensor_scalar_mul(mu[:], st_sum[:], 1.0 / D)
                m2 = ln.tile([1, T], F32, tag="m2")
                nc.vector.tensor_scalar_mul(m2[:], st_sq[:], 1.0 / D)
                mu2 = ln.tile([1, T], F32, tag="mu2")
                nc.vector.tensor_tensor(mu2[:], mu[:], mu[:], MULT)
                varE = ln.tile([1, T], F32, tag="varE")
                nc.vector.tensor_tensor(varE[:], m2[:], mu2[:], SUB)
                nc.vector.tensor_scalar_add(varE[:], varE[:], EPS_EFF)
                std = ln.tile([1, T], F32, tag="std")
                nc.scalar.activation(std[:], varE[:], SQRT)
                r0 = ln.tile([1, T], F32, tag="r0")
                nc.vector.reciprocal(r0[:], std[:])
                # one Newton step: r1 = r0 * (1.5 - 0.5 * varE * r0^2)
                nt1 = ln.tile([1, T], F32, tag="nt1")
                nc.vector.tensor_tensor(nt1[:], r0[:], r0[:], MULT)
                nc.vector.tensor_tensor(nt1[:], nt1[:], varE[:], MULT)
                nc.vector.tensor_scalar(nt1[:], nt1[:], -0.5, 1.5, MULT, ADD)
                rstd = ln.tile([1, T], F32R, tag="rstd")
                nc.vector.tensor_tensor(rstd[:], r0[:], nt1[:], MULT)
                mu_r = ln.tile([1, T], F32R, tag="mu_r")
                nc.vector.tensor_copy(mu_r[:], mu[:])

                ps_mu = ps_ln.tile([128, T], F32, tag="ps_mu")
                ps_r = ps_ln.tile([128, T], F32, tag="ps_r")
                nc.tensor.matmul(ps_mu[:], ones_row[:], mu_r[:], start=True, stop=True)
                nc.tensor.matmul(ps_r[:], ones_row[:], rstd[:], start=True, stop=True)

                for kc in range(KC):
                    t1 = ln.tile([128, T], F32, tag="t1")
                    nc.vector.tensor_tensor(t1[:], gatedT[:, kc, :].bitcast(F32), ps_mu[:], SUB)
                    nc.vector.tensor_tensor(t1[:], t1[:], ps_r[:], MULT)
                    nc.vector.tensor_scalar(normedT[:, kc, :], t1[:],
                                            gam[:, kc:kc + 1], bet[:, kc:kc + 1], MULT, ADD)

            # ================= stage 4: f2 + bias + store =================
            with (
                tc.tile_pool(name="w2pool", bufs=4) as w2pool,
                tc.tile_pool(name="yout", bufs=2) as yout,
                tc.tile_pool(name="ps_y", bufs=1, space="PSUM") as ps_y,
            ):
                for nf in range(2):
                    psy = [ps_y.tile([128, 512], F32, tag=f"y{tt}", name=f"psy{tt}") for tt in range(NT)]
                    for kc in range(KC):
                        w2b = w2pool.tile([128, 512], F32R, tag="w2b")
                        nc.sync.dma_start(
                            w2b[:], W2[kc * 128:(kc + 1) * 128, nf * 512:(nf + 1) * 512])
                        for tt in range(NT):
                            nc.tensor.matmul(psy[tt][:], normedT[:, kc, tt * 128:(tt + 1) * 128],
                                             w2b[:], start=(kc == 0), stop=(kc == KC - 1))
                    for tt in range(NT):
                        yo = yout.tile([128, 512], F32, tag="yo")
                        nc.vector.tensor_tensor(yo[:], psy[tt][:],
                                                b2_sb[:, nf * 512:(nf + 1) * 512], ADD)
                        nc.sync.dma_start(
                            y_s[tt * 128:(tt + 1) * 128, nf * 512:(nf + 1) * 512], yo[:])

    nc.compile()
    return nc


def _get_nc():
    if "nc" not in _CACHE:
        _CACHE["nc"] = _build()
    return _CACHE["nc"]


def kernel(x, W1, b1, W2, b2, gamma, beta, **kw):
    nc = _get_nc()
    x = np.ascontiguousarray(x, dtype=np.float32)
    in_maps = []
    for c in range(8):
        b = c // 4
        t0 = (c % 4) * T
        in_maps.append({
            "x_s": np.ascontiguousarray(x[b, t0:t0 + T, :]),
            "W1": np.ascontiguousarray(W1, dtype=np.float32),
            "b1": np.ascontiguousarray(b1, dtype=np.float32),
            "W2": np.ascontiguousarray(W2, dtype=np.float32),
            "b2": np.ascontiguousarray(b2, dtype=np.float32),
            "gamma": np.ascontiguousarray(gamma, dtype=np.float32),
            "beta": np.ascontiguousarray(beta, dtype=np.float32),
        })
    res = run_bass_kernel_spmd(nc, in_maps, core_ids=list(range(8)), **kw)
    y = np.empty((B, S, D), dtype=np.float32)
    for c in range(8):
        b = c // 4
        t0 = (c % 4) * T
        y[b, t0:t0 + T, :] = res.results[c]["y_s"]
    if kw:
        _CACHE["last_res"] = res
    return y



# revision 5
# speedup vs baseline: 1.5534x; 1.5534x over previous
"""HSTU block kernel for 8 Trainium2 NeuronCores — v2.

Sharding: token-parallel. Core c handles batch b=c//4, tokens
[(c%4)*512, (c%4+1)*512). k/v for the full 2048-token batch are exchanged
with two AllGathers (k bf16, v fp8) per 4-core group, issued right after
the k/v half of f1 so they overlap the q/u half.

Dataflow: f1 runs token-major (xT chunks stationary, W1 moving, all bf16,
biases injected as rank-1 ones x b1 matmuls into PSUM). q/k/u are
PE-transposed to feature-major for attention; v stays token-major and is
quantized to fp8e4m3 by the silu activation. Scores are bf16 matmuls
(K=64 per head); silu writes attention weights as fp8; AV runs as fp8
DoubleRow matmuls (ktc-pair contraction, out [64,512] per head) at double
rate. AV partials are accumulated in SBUF by the vector engine. LayerNorm
uses ones-matmul stats + Newton rsqrt; silu's /S scaling is folded into
eps' = S^2*eps; gamma/beta are folded into W2/b2 on the host. f2 runs
token-major (normedT stationary, W2 moving) across all 8 PSUM banks.
"""

import sys

sys.path.insert(0, "/opt/trn_rl_repo")

import ml_dtypes
import numpy as np

import concourse.bass as bass
import concourse.mybir as mybir
import concourse.tile as tile
from concourse import bacc
from concourse.bass_utils import run_bass_kernel_spmd
from concourse.masks import make_identity

F32 = mybir.dt.float32
BF16 = mybir.dt.bfloat16
FP8 = mybir.dt.float8e4
SILU = mybir.ActivationFunctionType.Silu
SQRT = mybir.ActivationFunctionType.Sqrt
MULT = mybir.AluOpType.mult
ADD = mybir.AluOpType.add
SUB = mybir.AluOpType.subtract
DR = mybir.MatmulPerfMode.DoubleRow

B, S, D = 2, 2048, 1024
T = 512            # tokens per core
NT = T // 128      # 4 token tiles per core
KC = D // 128      # 8 feature chunks
HP = 8             # head pairs (2 heads of dh=64 each)
NKC = S // 128     # 16 key-token chunks (full batch)
EPS_EFF = float(S) * float(S) * 1e-5

# W1 column blocks: u [0:D], v [D:2D], q [2D:3D], k [3D:4D]
U0, V0, Q0, K0 = 0, D, 2 * D, 3 * D

_CACHE = {}


def _build():
    nc = bacc.Bacc(None, target_bir_lowering=False, num_devices=8)

    x_s = nc.dram_tensor("x_s", [T, D], BF16, kind="ExternalInput")
    W1 = nc.dram_tensor("W1", [D, 4 * D], BF16, kind="ExternalInput")
    b1 = nc.dram_tensor("b1", [4 * D], BF16, kind="ExternalInput")
    W2 = nc.dram_tensor("W2", [D, D], BF16, kind="ExternalInput")
    b2 = nc.dram_tensor("b2", [D], BF16, kind="ExternalInput")
    y_s = nc.dram_tensor("y_s", [T, D], F32, kind="ExternalOutput")

    with tile.TileContext(nc) as tc:
        with (
            tc.tile_pool(name="persist", bufs=1) as sbp,
            tc.tile_pool(name="dram", bufs=1, space="DRAM") as dram,
        ):
            # ---- constants
            ident = sbp.tile([128, 128], BF16)
            make_identity(nc, ident[:])
            ones_row = sbp.tile([1, 128], BF16)
            nc.vector.memset(ones_row[:], 1.0)
            ones_col = sbp.tile([128, 1], BF16)
            nc.vector.memset(ones_col[:], 1.0)
            b1_sb = sbp.tile([1, 4 * D], BF16)
            nc.scalar.dma_start(b1_sb[:], b1[:][None, :])
            b2_row = sbp.tile([1, D], BF16)
            nc.scalar.dma_start(b2_row[:], b2[:][None, :])

            # ---- persistent activations
            xT = sbp.tile([128, KC, T], BF16)       # x^T (features on partitions)
            qT = sbp.tile([128, HP, T], BF16)
            uT = sbp.tile([128, HP, T], BF16)
            kT_loc = sbp.tile([128, HP, T], BF16)
            v_loc = sbp.tile([128, NT, D], FP8)     # token-major local v
            kT = sbp.tile([128, HP, S], BF16)       # full k, feature-major
            vF = sbp.tile([128, NKC, D], FP8)       # full v, token-major
            gatedT = sbp.tile([128, KC, T], BF16)
            sqT = sbp.tile([128, KC, T], BF16)
            normedT = sbp.tile([128, KC, T], BF16)
            W2_sb = sbp.tile([128, KC, D], BF16)
            b2_b = sbp.tile([128, D], F32)
            r_b = sbp.tile([128, T], F32)
            mur_b = sbp.tile([128, T], F32)

            # AG bounce buffers (v bitcast to bf16 for the collective)
            kv_in_k = dram.tile([128, HP, T], BF16)
            kv_out_k = dram.tile([512, HP, T], BF16)
            kv_in_v = dram.tile([128, NT, D // 2], BF16)
            kv_out_v = dram.tile([512, NT, D // 2], BF16)

            # ================= stage 0: load + transpose x =================
            with (
                tc.tile_pool(name="xload", bufs=2) as xload,
                tc.tile_pool(name="ps_tr", bufs=2, space="PSUM") as ps_tr,
            ):
                for tt in range(NT):
                    xa = xload.tile([128, D], BF16, tag="xa")
                    nc.sync.dma_start(xa[:], x_s[tt * 128:(tt + 1) * 128, :])
                    tr = ps_tr.tile([128, KC, 128], BF16, tag="tr")
                    for kc in range(KC):
                        nc.tensor.transpose(tr[:, kc, :], xa[:, kc * 128:(kc + 1) * 128],
                                            ident[:])
                    nc.vector.tensor_copy(xT[:, :, tt * 128:(tt + 1) * 128], tr[:])

                # ================= stage 1: f1 =================
                with (
                    tc.tile_pool(name="w1pool", bufs=1) as w1pool,
                    tc.tile_pool(name="tokpool", bufs=2) as tokpool,
                    tc.tile_pool(name="ps_f1", bufs=3, space="PSUM") as ps_f1,
                ):
                    # ---- pass A: v + k for all local tokens
                    wkv = w1pool.tile([128, KC, 2048], BF16, tag="w")
                    for fc in range(KC):
                        nc.sync.dma_start(wkv[:, fc, 0:1024],
                                          W1[fc * 128:(fc + 1) * 128, V0:V0 + D])
                        nc.sync.dma_start(wkv[:, fc, 1024:2048],
                                          W1[fc * 128:(fc + 1) * 128, K0:K0 + D])

                    for tt in range(NT):
                        ts = slice(tt * 128, (tt + 1) * 128)
                        # v sub-round
                        psv = ps_f1.tile([128, 2, T], F32, tag="f1")
                        for nf in range(2):
                            nc.tensor.matmul(psv[:, nf, :], ones_row[:],
                                             b1_sb[0:1, V0 + nf * 512:V0 + (nf + 1) * 512],
                                             start=True, stop=False)
                        for fc in range(KC):
                            for nf in range(2):
                                nc.tensor.matmul(psv[:, nf, :], xT[:, fc, ts],
                                                 wkv[:, fc, nf * 512:(nf + 1) * 512],
                                                 start=False, stop=(fc == KC - 1))
                        nc.scalar.activation(v_loc[:, tt, :],
                                             psv[:].rearrange("p a b -> p (a b)"), SILU)
                        # k sub-round
                        psk = ps_f1.tile([128, 2, T], F32, tag="f1")
                        for nf in range(2):
                            nc.tensor.matmul(psk[:, nf, :], ones_row[:],
                                             b1_sb[0:1, K0 + nf * 512:K0 + (nf + 1) * 512],
                                             start=True, stop=False)
                        for fc in range(KC):
                            for nf in range(2):
                                nc.tensor.matmul(psk[:, nf, :], xT[:, fc, ts],
                                                 wkv[:, fc, 1024 + nf * 512:1024 + (nf + 1) * 512],
                                                 start=False, stop=(fc == KC - 1))
                        k_tok = tokpool.tile([128, D], BF16, tag="ktok")
                        nc.scalar.activation(k_tok[:],
                                             psk[:].rearrange("p a b -> p (a b)"), SILU)
                        trk = ps_tr.tile([128, KC, 128], BF16, tag="tr")
                        for hc in range(KC):
                            nc.tensor.transpose(trk[:, hc, :],
                                                k_tok[:, hc * 128:(hc + 1) * 128], ident[:])
                        nc.vector.tensor_copy(kT_loc[:, :, ts], trk[:])

                    # ---- AllGather v then k (overlaps pass B)
                    nc.gpsimd.dma_start(kv_in_v[:], v_loc[:].bitcast(BF16))
                    nc.gpsimd.collective_compute(
                        "AllGather", mybir.AluOpType.bypass,
                        replica_groups=[[0, 1, 2, 3], [4, 5, 6, 7]],
                        ins=[kv_in_v[:]], outs=[kv_out_v[:]])
                    nc.gpsimd.dma_start(kv_in_k[:], kT_loc[:])
                    nc.gpsimd.collective_compute(
                        "AllGather", mybir.AluOpType.bypass,
                        replica_groups=[[0, 1, 2, 3], [4, 5, 6, 7]],
                        ins=[kv_in_k[:]], outs=[kv_out_k[:]])

                    # ---- pass B: q + u
                    wqu = w1pool.tile([128, KC, 2048], BF16, tag="w")
                    for fc in range(KC):
                        nc.sync.dma_start(wqu[:, fc, 0:1024],
                                          W1[fc * 128:(fc + 1) * 128, Q0:Q0 + D])
                        nc.sync.dma_start(wqu[:, fc, 1024:2048],
                                          W1[fc * 128:(fc + 1) * 128, U0:U0 + D])

                    for tt in range(NT):
                        ts = slice(tt * 128, (tt + 1) * 128)
                        psq = ps_f1.tile([128, 2, T], F32, tag="f1")
                        for nf in range(2):
                            nc.tensor.matmul(psq[:, nf, :], ones_row[:],
                                             b1_sb[0:1, Q0 + nf * 512:Q0 + (nf + 1) * 512],
                                             start=True, stop=False)
                        for fc in range(KC):
                            for nf in range(2):
                                nc.tensor.matmul(psq[:, nf, :], xT[:, fc, ts],
                                                 wqu[:, fc, nf * 512:(nf + 1) * 512],
                                                 start=False, stop=(fc == KC - 1))
                        q_tok = tokpool.tile([128, D], BF16, tag="qtok")
                        nc.scalar.activation(q_tok[:],
                                             psq[:].rearrange("p a b -> p (a b)"), SILU)
                        trq = ps_tr.tile([128, KC, 128], BF16, tag="tr")
                        for hc in range(KC):
                            nc.tensor.transpose(trq[:, hc, :],
                                                q_tok[:, hc * 128:(hc + 1) * 128], ident[:])
                        nc.vector.tensor_copy(qT[:, :, ts], trq[:])

                        psu = ps_f1.tile([128, 2, T], F32, tag="f1")
                        for nf in range(2):
                            nc.tensor.matmul(psu[:, nf, :], ones_row[:],
                                             b1_sb[0:1, U0 + nf * 512:U0 + (nf + 1) * 512],
                                             start=True, stop=False)
                        for fc in range(KC):
                            for nf in range(2):
                                nc.tensor.matmul(psu[:, nf, :], xT[:, fc, ts],
                                                 wqu[:, fc, 1024 + nf * 512:1024 + (nf + 1) * 512],
                                                 start=False, stop=(fc == KC - 1))
                        u_tok = tokpool.tile([128, D], BF16, tag="qtok")
                        nc.scalar.activation(u_tok[:],
                                             psu[:].rearrange("p a b -> p (a b)"), SILU)
                        tru = ps_tr.tile([128, KC, 128], BF16, tag="tr")
                        for hc in range(KC):
                            nc.tensor.transpose(tru[:, hc, :],
                                                u_tok[:, hc * 128:(hc + 1) * 128], ident[:])
                        nc.vector.tensor_copy(uT[:, :, ts], tru[:])

            # ---- W2 prefetch (no deps; lands during attention)
            for fc in range(KC):
                nc.sync.dma_start(W2_sb[:, fc, :], W2[fc * 128:(fc + 1) * 128, :])

            # ---- kv readback (vector queue; waits on the AllGathers)
            for r in range(4):
                nc.gpsimd.dma_start(vF[:, r * 4:(r + 1) * 4, :].bitcast(BF16),
                                    kv_out_v[r * 128:(r + 1) * 128, :, :])
            for r in range(4):
                nc.gpsimd.dma_start(kT[:, :, r * 512:(r + 1) * 512],
                                    kv_out_k[r * 128:(r + 1) * 128, :, :])

            # ================= stage 2: attention =================
            with (
                tc.tile_pool(name="apool", bufs=3) as apool,
                tc.tile_pool(name="avsb", bufs=2) as avsb,
                tc.tile_pool(name="ps_s", bufs=1, space="PSUM") as ps_s,
            ):
                for hp in range(HP):
                    # av for both heads accumulates at partitions 0:64 (DoubleRow
                    # dst must start at partition 0), head on the free axis
                    avp = ps_s.tile([128, 2, T], F32, tag="av", bufs=2, name=f"avp{hp}")
                    for j in range(NKC // 2):          # ktc pairs
                        k0, k1 = 2 * j, 2 * j + 1
                        for h in range(2):             # head within pair
                            hs = slice(64 * h, 64 * h + 64)
                            sp = ps_s.tile([128, 2, T], F32, tag="s", bufs=2,
                                           name=f"sp{hp}_{j}_{h}")
                            nc.tensor.matmul(sp[:, 0, :], kT[hs, hp, k0 * 128:(k0 + 1) * 128],
                                             qT[hs, hp, :], start=True, stop=True)
                            nc.tensor.matmul(sp[:, 1, :], kT[hs, hp, k1 * 128:(k1 + 1) * 128],
                                             qT[hs, hp, :], start=True, stop=True)
                            a = apool.tile([128, 2, T], FP8, tag="a", name=f"a{hp}_{j}_{h}")
                            nc.scalar.activation(a[:].rearrange("p a b -> p (a b)"),
                                                 sp[:].rearrange("p a b -> p (a b)"), SILU)
                            nc.tensor.matmul(avp[0:64, h, :],
                                             vF[:, k0:k0 + 2, hp * 128 + 64 * h:hp * 128 + 64 * h + 64],
                                             a[:], start=(j == 0), stop=(j == NKC // 2 - 1),
                                             perf_mode=DR)
                    av_sb = avsb.tile([64, 2, T], F32, tag="avsb", name=f"avsb{hp}")
                    nc.vector.tensor_copy(av_sb[:], avp[0:64, :, :])
                    avs = avsb.tile([128, T], F32, tag="avs", name=f"avs{hp}")
                    nc.gpsimd.dma_start(avs[0:64, :], av_sb[:, 0, :])
                    nc.gpsimd.dma_start(avs[64:128, :], av_sb[:, 1, :])
                    nc.vector.tensor_tensor(gatedT[:, hp, :], avs[:], uT[:, hp, :], MULT)
                    nc.vector.tensor_tensor(sqT[:, hp, :], gatedT[:, hp, :],
                                            gatedT[:, hp, :], MULT)

            # ================= stage 3: LayerNorm =================
            with (
                tc.tile_pool(name="ln", bufs=1) as ln,
                tc.tile_pool(name="ps_ln", bufs=1, space="PSUM") as ps_ln,
            ):
                st_sum = ps_ln.tile([1, T], F32, tag="st_sum")
                st_sq = ps_ln.tile([1, T], F32, tag="st_sq")
                for kc in range(KC):
                    nc.tensor.matmul(st_sum[:], ones_col[:], gatedT[:, kc, :],
                                     start=(kc == 0), stop=(kc == KC - 1))
                for kc in range(KC):
                    nc.tensor.matmul(st_sq[:], ones_col[:], sqT[:, kc, :],
                                     start=(kc == 0), stop=(kc == KC - 1))

                mu = ln.tile([1, T], F32, tag="mu")
                nc.vector.tensor_scalar_mul(mu[:], st_sum[:], 1.0 / D)
                m2 = ln.tile([1, T], F32, tag="m2")
                nc.vector.tensor_scalar_mul(m2[:], st_sq[:], 1.0 / D)
                mu2 = ln.tile([1, T], F32, tag="mu2")
                nc.vector.tensor_tensor(mu2[:], mu[:], mu[:], MULT)
                varE = ln.tile([1, T], F32, tag="varE")
                nc.vector.tensor_tensor(varE[:], m2[:], mu2[:], SUB)
                nc.vector.tensor_scalar_add(varE[:], varE[:], EPS_EFF)
                std = ln.tile([1, T], F32, tag="std")
                nc.scalar.activation(std[:], varE[:], SQRT)
                r0 = ln.tile([1, T], F32, tag="r0")
                nc.vector.reciprocal(r0[:], std[:])
                # one Newton step: r1 = r0 * (1.5 - 0.5 * varE * r0^2)
                nt1 = ln.tile([1, T], F32, tag="nt1")
                nc.vector.tensor_tensor(nt1[:], r0[:], r0[:], MULT)
                nc.vector.tensor_tensor(nt1[:], nt1[:], varE[:], MULT)
                nc.vector.tensor_scalar(nt1[:], nt1[:], -0.5, 1.5, MULT, ADD)
                rstd = ln.tile([1, T], BF16, tag="rstd")
                nc.vector.tensor_tensor(rstd[:], r0[:], nt1[:], MULT)
                murs = ln.tile([1, T], BF16, tag="murs")
                nc.vector.tensor_tensor(murs[:], rstd[:], mu[:], MULT)

                ps_r = ps_ln.tile([128, T], F32, tag="ps_r")
                ps_mu = ps_ln.tile([128, T], F32, tag="ps_mu")
                nc.tensor.matmul(ps_r[:], ones_row[:], rstd[:], start=True, stop=True)
                nc.tensor.matmul(ps_mu[:], ones_row[:], murs[:], start=True, stop=True)
                nc.vector.tensor_copy(r_b[:], ps_r[:])
                nc.vector.tensor_copy(mur_b[:], ps_mu[:])

                # b2 broadcast while banks are free
                ps_b2 = ps_ln.tile([128, 2, 512], F32, tag="ps_b2")
                for nf in range(2):
                    nc.tensor.matmul(ps_b2[:, nf, :], ones_row[:],
                                     b2_row[0:1, nf * 512:(nf + 1) * 512],
                                     start=True, stop=True)
                nc.vector.tensor_copy(b2_b[:], ps_b2[:].rearrange("p a b -> p (a b)"))

                # normed = gated * rstd - mu * rstd
                for kc in range(KC):
                    t1 = ln.tile([128, T], F32, tag="t1", bufs=2)
                    nc.vector.tensor_tensor(t1[:], gatedT[:, kc, :], r_b[:], MULT)
                    nc.vector.tensor_tensor(normedT[:, kc, :], t1[:], mur_b[:], SUB)

            # ================= stage 4: f2 + bias + store =================
            with (
                tc.tile_pool(name="yout", bufs=2) as yout,
                tc.tile_pool(name="ps_y", bufs=1, space="PSUM") as ps_y,
            ):
                psy = [ps_y.tile([128, 512], F32, tag=f"y{i}", name=f"psy{i}")
                       for i in range(8)]
                for fc in range(KC):
                    for tt in range(NT):
                        for nf in range(2):
                            nc.tensor.matmul(psy[tt * 2 + nf][:],
                                             normedT[:, fc, tt * 128:(tt + 1) * 128],
                                             W2_sb[:, fc, nf * 512:(nf + 1) * 512],
                                             start=(fc == 0), stop=(fc == KC - 1))
                for tt in range(NT):
                    for nf in range(2):
                        yo = yout.tile([128, 512], F32, tag="yo")
                        nc.vector.tensor_tensor(yo[:], psy[tt * 2 + nf][:],
                                                b2_b[:, nf * 512:(nf + 1) * 512], ADD)
                        nc.scalar.dma_start(
                            y_s[tt * 128:(tt + 1) * 128, nf * 512:(nf + 1) * 512], yo[:])

    nc.compile()
    return nc


def _get_nc():
    if "nc" not in _CACHE:
        _CACHE["nc"] = _build()
    return _CACHE["nc"]


def kernel(x, W1, b1, W2, b2, gamma, beta, **kw):
    nc = _get_nc()
    x = np.asarray(x, dtype=np.float32)
    W1b = np.ascontiguousarray(np.asarray(W1, dtype=np.float32).astype(ml_dtypes.bfloat16))
    b1b = np.ascontiguousarray(np.asarray(b1, dtype=np.float32).astype(ml_dtypes.bfloat16))
    # fold gamma/beta into W2/b2
    gamma = np.asarray(gamma, dtype=np.float64)
    beta = np.asarray(beta, dtype=np.float64)
    W2f = np.asarray(W2, dtype=np.float64)
    b2f = np.asarray(b2, dtype=np.float64)
    W2p = np.ascontiguousarray((gamma[:, None] * W2f).astype(ml_dtypes.bfloat16))
    b2p = np.ascontiguousarray((beta @ W2f + b2f).astype(ml_dtypes.bfloat16))
    xb = x.astype(ml_dtypes.bfloat16)
    in_maps = []
    for c in range(8):
        b = c // 4
        t0 = (c % 4) * T
        in_maps.append({
            "x_s": np.ascontiguousarray(xb[b, t0:t0 + T, :]),
            "W1": W1b,
            "b1": b1b,
            "W2": W2p,
            "b2": b2p,
        })
    res = run_bass_kernel_spmd(nc, in_maps, core_ids=list(range(8)), **kw)
    y = np.empty((B, S, D), dtype=np.float32)
    for c in range(8):
        b = c // 4
        t0 = (c % 4) * T
        y[b, t0:t0 + T, :] = res.results[c]["y_s"]
    if kw:
        _CACHE["last_res"] = res
    return y
